# revision 42
# baseline (speedup 1.0000x reference)
"""TRN2 Bass kernel for nn_Construct_76484777607483.

Computes, for 12 input tensors x_i [B=2, C=256, H=64, W=256]:
    y_i = einsum('bchw,co->bohw', x_i, W)
interleaved over H (output row 12*h + i comes from tensor i, row h) into
out [2, 256, 768, 256], plus bias b[o] * count(row) where count is the
conv-transpose overlap multiplicity (ramp 1..12 at the top edge, 12 in the
middle, 12..1 at the bottom edge).

Sharding: 8 cores = (2 batches) x (4 h-quarters of 16 input rows).

Datapath is fp16 end-to-end (inputs cast on host, outputs stored fp16 and
upcast on host; fp16 over bf16 for its 8x lower rounding error — values are
well inside fp16 range): fp16 matmul runs at the same 1 cycle/row PE rate as fp32r
but halves every DMA. Queue roles: SP ring carries all input DMA (75.8us),
gpsimd/SWDGE ring all output DMA (75.8us), ACT evacuates mh=0 PSUM tiles
(bias-add via activation Identity), DVE evacuates mh=1 (tensor_scalar_add);
PE at 81.9us is the bottleneck, as it should be for this compute-regime
problem.

Per (stripe s of 4 input rows, tensor i): one input DMA [128, 2kh, 1024],
two PSUM tiles (mh halves) of [128, 4, 256] each built by accumulating
fp16 matmuls in 512-element chunks (the ISA's per-matmul PSUM-bank limit),
evacuated with the per-(i,row) bias b[o]*count added as a per-partition
scalar, then one output DMA per mh into y[mh, :, hl, i, :]
whose (hl, i) index order IS the row interleave, so the host just reshapes.
Edge stripes (s=0 row 0, s=3 row 3) split the evac in two because the bias
count varies on the outermost output rows; the split is structural on all
cores (SPMD), only the bias table data differs.
"""

import numpy as np

import concourse.bacc as bacc
import concourse.tile as tile
import concourse.mybir as mybir
from concourse.bass_utils import run_bass_kernel_spmd

B, C, H, WD = 2, 256, 64, 256
NT = 12                 # stacked tensors
NCORES = 8
HQ = H // 4             # 16 input rows per core
NS = 4                  # stripes per core
SR = HQ // NS           # 4 input rows per stripe
HOUT = NT * H           # 768

_F32 = mybir.dt.float32
_DT16 = mybir.dt.float16
_NP16 = mybir.dt.np(_DT16)

_NC_CACHE = {}


def build_nc(n_warm=16, first_split=True, xin_bufs=6, ob_bufs=8, ps_bufs=4, last_split=True,
             sp_tail_n=1, slb_ring="sp", mh_swap=False):
    key = (n_warm, first_split, xin_bufs, ob_bufs, ps_bufs, last_split, sp_tail_n, slb_ring, mh_swap)
    if key in _NC_CACHE:
        return _NC_CACHE[key]
    nc = bacc.Bacc("TRN2", target_bir_lowering=False)
    # x[p, s, i, kh, r*WD]: channel = kh*128 + p, input row = s*SR + r
    x_d = nc.declare_dram_parameter("x", [128, NS, NT, 2, SR * WD], _DT16, isOutput=False)
    w_d = nc.declare_dram_parameter("w", [2, 128, 2 * 128], _DT16, isOutput=False)
    bv_d = nc.declare_dram_parameter("bv", [2, 128, NT * HQ], _F32, isOutput=False)
    # y[mh, p, hl, i, w]: output channel = mh*128 + p, local out row = hl*NT + i
    y_d = nc.declare_dram_parameter("y", [2, 128, HQ, NT, WD], _DT16, isOutput=True)

    with tile.TileContext(nc) as tc:
        with (
            tc.tile_pool(name="const", bufs=1) as cpool,
            tc.tile_pool(name="xin", bufs=xin_bufs) as inpool,
            tc.tile_pool(name="obuf", bufs=ob_bufs) as outpool,
            tc.tile_pool(name="ps", bufs=ps_bufs, space="PSUM") as pspool,
        ):
            # consts all on the gpsimd ring, which is otherwise idle at the
            # head (ACT's head is occupied by the auto-inserted activation
            # table load; SP must start input tiles immediately); W per-kh,
            # kh0 first since it gates the first matmul
            # (the stationary matmul operand must be contiguous per partition,
            # so each [128,128] quadrant gets its own tile)
            wt = [
                [cpool.tile([128, 128], _DT16, name=f"w{kh}{mh}") for mh in range(2)]
                for kh in range(2)
            ]
            for mh in range(2):
                for kh in range(2):
                    nc.gpsimd.dma_start(
                        out=wt[kh][mh][:], in_=w_d[kh, :, mh * 128 : (mh + 1) * 128]
                    )
            # bias table on ACT (behind its table load, ready before the
            # first evac); keeping it off Pool keeps the W sems early, and
            # a late bv sem would stall the first evac -> PSUM recycling -> PE
            bvt = [cpool.tile([128, NT * HQ], _F32, name=f"bv{mh}") for mh in range(2)]
            for mh in range(2):
                nc.scalar.dma_start(out=bvt[mh][:], in_=bv_d[mh])

            # PE p-state warmup: pe_busy_start is sticky, so a burst of tiny
            # matmuls during the input-DMA fill window starts the 3us clock
            # ramp early and the real matmuls all run at full rate
            if n_warm:
                wscr = cpool.tile([128, 128], _DT16, name="wscr")
                zscr = cpool.tile([128, 64], _DT16, name="zscr")
                nc.vector.memset(wscr[:], 0.0)
                nc.vector.memset(zscr[:], 0.0)
                warm = pspool.tile([128, SR, WD], _F32, name="warm", tag="ps")
                for _ in range(n_warm):
                    nc.tensor.matmul(warm[:, 0, 0:64], wscr[:], zscr[:], start=True, stop=True)

            slb = {"sp": nc.sync, "act": nc.scalar, "pool": nc.gpsimd}[slb_ring]
            for s in range(NS):
                for i in range(NT):
                    if first_split and s == 0 and i == 0:
                        # first iteration: each kh half in its OWN tile so the
                        # first matmul waits only on its 790ns half-load (tile
                        # dependencies are tile-granular, and each DMA's
                        # completion sem costs +900ns)
                        xk = [
                            inpool.tile([128, SR * WD], _DT16, name=f"xk{kh}", tag=f"xk{kh}")
                            for kh in range(2)
                        ]
                        for kh in range(2):
                            nc.sync.dma_start(out=xk[kh][:], in_=x_d[:, s, i, kh])
                        xsl = lambda kh, a, b: xk[kh][:, a:b]
                    else:
                        xin = inpool.tile([128, 2, SR * WD], _DT16, name=f"x{s}_{i}", tag="xin")
                        nc.sync.dma_start(out=xin[:], in_=x_d[:, s, i])
                        xsl = lambda kh, a, b: xin[:, kh, a:b]
                    last_iter = s == NS - 1 and i == NT - 1
                    for mh in ((1, 0) if (mh_swap or last_iter) else (0, 1)):
                        ps = pspool.tile([128, SR, WD], _F32, name=f"ps{s}_{i}_{mh}", tag="ps")
                        # bias b[o]*count as per-partition scalar; count is
                        # uniform within an op, so edge stripes split the
                        # boundary row off (count ramps on the outer 11 rows).
                        # s=NS-1 also splits the matmuls/DMA so the kernel's
                        # drain tail ends on a 1-row sliver
                        if s == 0:
                            mm_parts = [(0, SR)]
                            parts = [(0, 1, 0), (1, SR, 1)]
                        elif s == NS - 1:
                            mm_parts = [(0, SR - 1), (SR - 1, SR)]
                            parts = [(0, SR - 1, s * SR), (SR - 1, SR, HQ - 1)]
                        else:
                            mm_parts = [(0, SR)]
                            parts = [(0, SR, s * SR)]
                        # the ISA caps a matmul's moving/out free size at 512
                        # elements (one PSUM bank), so emit 2-row chunks
                        for r0, r1 in mm_parts:
                            for c0 in range(r0, r1, 2):
                                c1 = min(c0 + 2, r1)
                                for j, kh in enumerate((0, 1)):
                                    nc.tensor.matmul(
                                        ps[:, c0:c1], wt[kh][mh][:],
                                        xsl(kh, c0 * WD, c1 * WD),
                                        start=(j == 0), stop=(j == 1),
                                    )
                        ob = outpool.tile([128, SR, WD], _DT16, name=f"ob{s}_{i}_{mh}", tag=f"ob{mh}")
                        for r0, r1, hl in parts:
                            col = i * HQ + hl
                            if mh == 0:
                                nc.scalar.activation(
                                    ob[:, r0:r1],
                                    ps[:, r0:r1],
                                    mybir.ActivationFunctionType.Identity,
                                    bias=bvt[mh][:, col : col + 1],
                                )
                            else:
                                nc.vector.tensor_scalar_add(
                                    ob[:, r0:r1],
                                    ps[:, r0:r1],
                                    bvt[mh][:, col : col + 1],
                                )
                        if last_split and last_iter:
                            # tail: pipeline the final pieces across rings
                            # (the SP ring is drained by now and HWDGE has a
                            # shorter completion-sem lag than SWDGE)
                            ea = nc.scalar if mh == 0 else nc.gpsimd
                            eb = nc.sync if mh == 0 else slb
                            ea.dma_start(
                                out=y_d[mh, :, s * SR : s * SR + SR - 1, i, :],
                                in_=ob[:, 0 : SR - 1],
                            )
                            eb.dma_start(
                                out=y_d[mh, :, s * SR + SR - 1 : (s + 1) * SR, i, :],
                                in_=ob[:, SR - 1 : SR],
                            )
                        else:
                            # the gpsimd/SWDGE completion sem lags ~1.1us
                            # behind the transfer, so the tail-most regular
                            # outputs go on the drained SP ring instead
                            out_eng = (
                                nc.sync
                                if (last_split and s == NS - 1 and i >= NT - sp_tail_n)
                                else nc.gpsimd
                            )
                            out_eng.dma_start(
                                out=y_d[mh, :, s * SR : (s + 1) * SR, i, :],
                                in_=ob[:],
                            )
    nc.finalize()
    _NC_CACHE[key] = nc
    return nc


def _counts() -> np.ndarray:
    """count[r] for output row r (conv-transpose bias multiplicity)."""
    r = np.arange(HOUT)
    return (np.minimum(11, r) - np.maximum(0, r - (HOUT - NT)) + 1).astype(np.float32)


def shard_inputs(inputs: dict) -> list[dict]:
    xs = [np.asarray(inputs[f"x{i}"], dtype=np.float32) for i in range(NT)]
    w = np.asarray(inputs["W"], dtype=np.float32)
    b = np.asarray(inputs["b"], dtype=np.float32)
    counts = _counts()
    # w[kh, k, mh, m] = W[kh*128+k, mh*128+m]
    wp = np.ascontiguousarray(
        w.reshape(2, 128, 2 * 128).astype(_NP16)
    )
    in_maps = []
    for cid in range(NCORES):
        b_idx, hq = divmod(cid, 4)
        h0 = hq * HQ
        # x[p, s, i, kh, r*WD] = x_i[b, kh*128+p, h0+s*SR+r, w]
        xp = np.empty((128, NS, NT, 2, SR * WD), dtype=_NP16)
        for i in range(NT):
            blk = xs[i][b_idx, :, h0 : h0 + HQ, :]  # [256, 16, 256]
            blk = blk.reshape(2, 128, NS, SR * WD)  # [kh, p, s, rw]
            xp[:, :, i] = blk.transpose(1, 2, 0, 3).astype(_NP16)
        # bv[mh, m, i*HQ + hl] = b[mh*128+m] * count(12*(h0+hl) + i)
        i_idx = np.arange(NT)[:, None]
        hl_idx = np.arange(HQ)[None, :]
        cnt = counts[NT * (h0 + hl_idx) + i_idx].reshape(NT * HQ)  # [192]
        bv = (b.reshape(2, 128)[:, :, None] * cnt[None, None, :]).astype(np.float32)
        in_maps.append({"x": xp, "w": wp, "bv": bv})
    return in_maps


def gather_outputs(results: list[dict]) -> np.ndarray:
    out = np.empty((B, C, HOUT, WD), dtype=np.float32)
    for cid in range(NCORES):
        b_idx, hq = divmod(cid, 4)
        h0 = hq * HQ
        # y[mh, p, hl, i, w] -> rows hl*NT+i: exactly the interleave order
        y = np.asarray(results[cid]["y"]).reshape(C, HQ * NT, WD)
        out[b_idx, :, NT * h0 : NT * (h0 + HQ), :] = y.astype(np.float32)
    return out


def kernel(**inputs) -> np.ndarray:
    nc = build_nc()
    in_maps = shard_inputs(inputs)
    res = run_bass_kernel_spmd(nc, in_maps, core_ids=list(range(NCORES)))
    return gather_outputs(res.results)


# revision 50
# speedup vs baseline: 1.0025x; 1.0025x over previous
"""TRN2 Bass kernel for nn_Construct_76484777607483.

Computes, for 12 input tensors x_i [B=2, C=256, H=64, W=256]:
    y_i = einsum('bchw,co->bohw', x_i, W)
interleaved over H (output row 12*h + i comes from tensor i, row h) into
out [2, 256, 768, 256], plus bias b[o] * count(row) where count is the
conv-transpose overlap multiplicity (ramp 1..12 at the top edge, 12 in the
middle, 12..1 at the bottom edge).

Sharding: 8 cores = (2 batches) x (4 h-quarters of 16 input rows).

Datapath is fp16 end-to-end (inputs cast on host, outputs stored fp16 and
upcast on host; fp16 over bf16 for its 8x lower rounding error — values are
well inside fp16 range): fp16 matmul runs at the same 1 cycle/row PE rate as fp32r
but halves every DMA. Queue roles: SP ring carries all input DMA (75.8us),
gpsimd/SWDGE ring all output DMA (75.8us), ACT evacuates mh=0 PSUM tiles
(bias-add via activation Identity), DVE evacuates mh=1 (tensor_scalar_add);
PE at 81.9us is the bottleneck, as it should be for this compute-regime
problem.

Per (stripe s of 4 input rows, tensor i): one input DMA [128, 2kh, 1024],
two PSUM tiles (mh halves) of [128, 4, 256] each built by accumulating
fp16 matmuls in 512-element chunks (the ISA's per-matmul PSUM-bank limit),
evacuated with the per-(i,row) bias b[o]*count added as a per-partition
scalar, then one output DMA per mh into y[mh, :, hl, i, :]
whose (hl, i) index order IS the row interleave, so the host just reshapes.
Edge stripes (s=0 row 0, s=3 row 3) split the evac in two because the bias
count varies on the outermost output rows; the split is structural on all
cores (SPMD), only the bias table data differs.
"""

import numpy as np

import concourse.bacc as bacc
import concourse.tile as tile
import concourse.mybir as mybir
from concourse.bass_utils import run_bass_kernel_spmd

B, C, H, WD = 2, 256, 64, 256
NT = 12                 # stacked tensors
NCORES = 8
HQ = H // 4             # 16 input rows per core
NS = 4                  # stripes per core
SR = HQ // NS           # 4 input rows per stripe
HOUT = NT * H           # 768

_F32 = mybir.dt.float32
_DT16 = mybir.dt.float16
_NP16 = mybir.dt.np(_DT16)

_NC_CACHE = {}


def build_nc(n_warm=16, first_split=True, xin_bufs=6, ob_bufs=8, ps_bufs=4, last_split=True,
             sp_tail_n=1, slb_ring="sp", mh_swap=False,
             mh0_3piece=False, sp_mh1_n=0, sp_mh0_n=0, p3r=("sp", "pool", "act"), split_ps=True, mh1b_act=True, m0r=("sp", "sp")):
    key = (n_warm, first_split, xin_bufs, ob_bufs, ps_bufs, last_split, sp_tail_n, slb_ring, mh_swap, mh0_3piece, sp_mh1_n, sp_mh0_n, p3r, split_ps, mh1b_act, m0r)
    if key in _NC_CACHE:
        return _NC_CACHE[key]
    nc = bacc.Bacc("TRN2", target_bir_lowering=False)
    # x[p, s, i, kh, r*WD]: channel = kh*128 + p, input row = s*SR + r
    x_d = nc.declare_dram_parameter("x", [128, NS, NT, 2, SR * WD], _DT16, isOutput=False)
    w_d = nc.declare_dram_parameter("w", [2, 128, 2 * 128], _DT16, isOutput=False)
    bv_d = nc.declare_dram_parameter("bv", [2, 128, NT * HQ], _F32, isOutput=False)
    # y[mh, p, hl, i, w]: output channel = mh*128 + p, local out row = hl*NT + i
    y_d = nc.declare_dram_parameter("y", [2, 128, HQ, NT, WD], _DT16, isOutput=True)

    with tile.TileContext(nc) as tc:
        with (
            tc.tile_pool(name="const", bufs=1) as cpool,
            tc.tile_pool(name="xin", bufs=xin_bufs) as inpool,
            tc.tile_pool(name="obuf", bufs=ob_bufs) as outpool,
            tc.tile_pool(name="ps", bufs=ps_bufs, space="PSUM") as pspool,
        ):
            # consts all on the gpsimd ring, which is otherwise idle at the
            # head (ACT's head is occupied by the auto-inserted activation
            # table load; SP must start input tiles immediately); W per-kh,
            # kh0 first since it gates the first matmul
            # (the stationary matmul operand must be contiguous per partition,
            # so each [128,128] quadrant gets its own tile)
            wt = [
                [cpool.tile([128, 128], _DT16, name=f"w{kh}{mh}") for mh in range(2)]
                for kh in range(2)
            ]
            for mh in range(2):
                for kh in range(2):
                    nc.gpsimd.dma_start(
                        out=wt[kh][mh][:], in_=w_d[kh, :, mh * 128 : (mh + 1) * 128]
                    )
            # bias table on ACT (behind its table load, ready before the
            # first evac); keeping it off Pool keeps the W sems early, and
            # a late bv sem would stall the first evac -> PSUM recycling -> PE
            bvt = [cpool.tile([128, NT * HQ], _F32, name=f"bv{mh}") for mh in range(2)]
            for mh in range(2):
                nc.scalar.dma_start(out=bvt[mh][:], in_=bv_d[mh])

            # PE p-state warmup: pe_busy_start is sticky, so a burst of tiny
            # matmuls during the input-DMA fill window starts the 3us clock
            # ramp early and the real matmuls all run at full rate
            if n_warm:
                wscr = cpool.tile([128, 128], _DT16, name="wscr")
                zscr = cpool.tile([128, 64], _DT16, name="zscr")
                nc.vector.memset(wscr[:], 0.0)
                nc.vector.memset(zscr[:], 0.0)
                warm = pspool.tile([128, SR, WD], _F32, name="warm", tag="ps")
                for _ in range(n_warm):
                    nc.tensor.matmul(warm[:, 0, 0:64], wscr[:], zscr[:], start=True, stop=True)

            slb = {"sp": nc.sync, "act": nc.scalar, "pool": nc.gpsimd}[slb_ring]
            for s in range(NS):
                for i in range(NT):
                    if first_split and s == 0 and i == 0:
                        # first iteration: each kh half in its OWN tile so the
                        # first matmul waits only on its 790ns half-load (tile
                        # dependencies are tile-granular, and each DMA's
                        # completion sem costs +900ns)
                        xk = [
                            inpool.tile([128, SR * WD], _DT16, name=f"xk{kh}", tag=f"xk{kh}")
                            for kh in range(2)
                        ]
                        for kh in range(2):
                            nc.sync.dma_start(out=xk[kh][:], in_=x_d[:, s, i, kh])
                        xsl = lambda kh, a, b: xk[kh][:, a:b]
                    else:
                        xin = inpool.tile([128, 2, SR * WD], _DT16, name=f"x{s}_{i}", tag="xin")
                        nc.sync.dma_start(out=xin[:], in_=x_d[:, s, i])
                        xsl = lambda kh, a, b: xin[:, kh, a:b]
                    last_iter = s == NS - 1 and i == NT - 1
                    for mh in ((1, 0) if (mh_swap or last_iter) else (0, 1)):
                        ps = pspool.tile([128, SR, WD], _F32, name=f"ps{s}_{i}_{mh}", tag="ps")
                        # evac deps are tile-granular (an evac waits for ALL
                        # matmul chunks of its PSUM tile), so the very last
                        # tile splits its boundary row into a second tile:
                        # the rows0-2 evac can then start before the final mm
                        psb = (
                            pspool.tile([128, SR, WD], _F32, name=f"psb{mh}", tag="ps")
                            if (split_ps and last_iter)
                            else None
                        )
                        # bias b[o]*count as per-partition scalar; count is
                        # uniform within an op, so edge stripes split the
                        # boundary row off (count ramps on the outer 11 rows).
                        # s=NS-1 also splits the matmuls/DMA so the kernel's
                        # drain tail ends on a 1-row sliver
                        if s == 0:
                            mm_parts = [(0, SR)]
                            parts = [(0, 1, 0), (1, SR, 1)]
                        elif s == NS - 1:
                            if mh0_3piece and last_iter and mh == 0:
                                mm_parts = [(0, SR - 1), (SR - 1, SR)]
                                parts = [(0, 2, s * SR), (2, 3, s * SR + 2), (SR - 1, SR, HQ - 1)]
                            else:
                                mm_parts = [(0, SR - 1), (SR - 1, SR)]
                                parts = [(0, SR - 1, s * SR), (SR - 1, SR, HQ - 1)]
                        else:
                            mm_parts = [(0, SR)]
                            parts = [(0, SR, s * SR)]
                        # the ISA caps a matmul's moving/out free size at 512
                        # elements (one PSUM bank), so emit 2-row chunks
                        for r0, r1 in mm_parts:
                            for c0 in range(r0, r1, 2):
                                c1 = min(c0 + 2, r1)
                                pdst = psb if (psb is not None and c0 >= SR - 1) else ps
                                for j, kh in enumerate((0, 1)):
                                    nc.tensor.matmul(
                                        pdst[:, c0:c1], wt[kh][mh][:],
                                        xsl(kh, c0 * WD, c1 * WD),
                                        start=(j == 0), stop=(j == 1),
                                    )
                        ob = outpool.tile([128, SR, WD], _DT16, name=f"ob{s}_{i}_{mh}", tag=f"ob{mh}")
                        for r0, r1, hl in parts:
                            col = i * HQ + hl
                            psrc = psb if (psb is not None and r0 >= SR - 1) else ps
                            # the last iteration's boundary-row evacs swap
                            # engines: mh1's goes to ACT (idle in that window)
                            # and mh0's to DVE (free by then), so neither
                            # queues behind the other tail evacs
                            if last_iter and r0 >= SR - 1 and mh1b_act:
                                use_act = mh == 1
                            else:
                                use_act = mh == 0
                            if use_act:
                                nc.scalar.activation(
                                    ob[:, r0:r1],
                                    psrc[:, r0:r1],
                                    mybir.ActivationFunctionType.Identity,
                                    bias=bvt[mh][:, col : col + 1],
                                )
                            else:
                                nc.vector.tensor_scalar_add(
                                    ob[:, r0:r1],
                                    psrc[:, r0:r1],
                                    bvt[mh][:, col : col + 1],
                                )
                        if last_split and last_iter:
                            # tail: pipeline the final pieces across rings
                            # (the SP ring is drained by now and HWDGE has a
                            # shorter completion-sem lag than SWDGE)
                            if mh0_3piece and mh == 0:
                                r3 = {"sp": nc.sync, "act": nc.scalar, "pool": nc.gpsimd}
                                pieces = [(0, 2, r3[p3r[0]]), (2, 3, r3[p3r[1]]), (3, 4, r3[p3r[2]])]
                            elif mh == 0:
                                r3 = {"sp": nc.sync, "act": nc.scalar, "pool": nc.gpsimd}
                                pieces = [(0, SR - 1, r3[m0r[0]]), (SR - 1, SR, r3[m0r[1]])]
                            else:
                                pieces = [(0, SR - 1, nc.gpsimd), (SR - 1, SR, slb)]
                            for r0, r1, eng in pieces:
                                eng.dma_start(
                                    out=y_d[mh, :, s * SR + r0 : s * SR + r1, i, :],
                                    in_=ob[:, r0:r1],
                                )
                        else:
                            # the gpsimd/SWDGE completion sem lags ~1.1us
                            # behind the transfer, so the tail-most regular
                            # outputs go on the drained SP ring instead
                            k = (sp_mh1_n if mh == 1 else sp_mh0_n)
                            out_eng = (
                                nc.sync
                                if (last_split and s == NS - 1
                                    and (i >= NT - sp_tail_n or i >= NT - 1 - k))
                                else nc.gpsimd
                            )
                            out_eng.dma_start(
                                out=y_d[mh, :, s * SR : (s + 1) * SR, i, :],
                                in_=ob[:],
                            )
    nc.finalize()
    _NC_CACHE[key] = nc
    return nc


def _counts() -> np.ndarray:
    """count[r] for output row r (conv-transpose bias multiplicity)."""
    r = np.arange(HOUT)
    return (np.minimum(11, r) - np.maximum(0, r - (HOUT - NT)) + 1).astype(np.float32)


def shard_inputs(inputs: dict) -> list[dict]:
    xs = [np.asarray(inputs[f"x{i}"], dtype=np.float32) for i in range(NT)]
    w = np.asarray(inputs["W"], dtype=np.float32)
    b = np.asarray(inputs["b"], dtype=np.float32)
    counts = _counts()
    # w[kh, k, mh, m] = W[kh*128+k, mh*128+m]
    wp = np.ascontiguousarray(
        w.reshape(2, 128, 2 * 128).astype(_NP16)
    )
    in_maps = []
    for cid in range(NCORES):
        b_idx, hq = divmod(cid, 4)
        h0 = hq * HQ
        # x[p, s, i, kh, r*WD] = x_i[b, kh*128+p, h0+s*SR+r, w]
        xp = np.empty((128, NS, NT, 2, SR * WD), dtype=_NP16)
        for i in range(NT):
            blk = xs[i][b_idx, :, h0 : h0 + HQ, :]  # [256, 16, 256]
            blk = blk.reshape(2, 128, NS, SR * WD)  # [kh, p, s, rw]
            xp[:, :, i] = blk.transpose(1, 2, 0, 3).astype(_NP16)
        # bv[mh, m, i*HQ + hl] = b[mh*128+m] * count(12*(h0+hl) + i)
        i_idx = np.arange(NT)[:, None]
        hl_idx = np.arange(HQ)[None, :]
        cnt = counts[NT * (h0 + hl_idx) + i_idx].reshape(NT * HQ)  # [192]
        bv = (b.reshape(2, 128)[:, :, None] * cnt[None, None, :]).astype(np.float32)
        in_maps.append({"x": xp, "w": wp, "bv": bv})
    return in_maps


def gather_outputs(results: list[dict]) -> np.ndarray:
    out = np.empty((B, C, HOUT, WD), dtype=np.float32)
    for cid in range(NCORES):
        b_idx, hq = divmod(cid, 4)
        h0 = hq * HQ
        # y[mh, p, hl, i, w] -> rows hl*NT+i: exactly the interleave order
        y = np.asarray(results[cid]["y"]).reshape(C, HQ * NT, WD)
        out[b_idx, :, NT * h0 : NT * (h0 + HQ), :] = y.astype(np.float32)
    return out


def kernel(**inputs) -> np.ndarray:
    nc = build_nc()
    in_maps = shard_inputs(inputs)
    res = run_bass_kernel_spmd(nc, in_maps, core_ids=list(range(NCORES)))
    return gather_outputs(res.results)


# revision 54
# speedup vs baseline: 1.0037x; 1.0012x over previous
"""TRN2 Bass kernel for nn_Construct_76484777607483.

Computes, for 12 input tensors x_i [B=2, C=256, H=64, W=256]:
    y_i = einsum('bchw,co->bohw', x_i, W)
interleaved over H (output row 12*h + i comes from tensor i, row h) into
out [2, 256, 768, 256], plus bias b[o] * count(row) where count is the
conv-transpose overlap multiplicity (ramp 1..12 at the top edge, 12 in the
middle, 12..1 at the bottom edge).

Sharding: 8 cores = (2 batches) x (4 h-quarters of 16 input rows).

Datapath is fp16 end-to-end (inputs cast on host, outputs stored fp16 and
upcast on host; fp16 over bf16 for its 8x lower rounding error — values are
well inside fp16 range): fp16 matmul runs at the same 1 cycle/row PE rate as fp32r
but halves every DMA. Queue roles: SP ring carries all input DMA (75.8us),
gpsimd/SWDGE ring all output DMA (75.8us), ACT evacuates mh=0 PSUM tiles
(bias-add via activation Identity), DVE evacuates mh=1 (tensor_scalar_add);
PE at 81.9us is the bottleneck, as it should be for this compute-regime
problem.

Per (stripe s of 4 input rows, tensor i): one input DMA [128, 2kh, 1024],
two PSUM tiles (mh halves) of [128, 4, 256] each built by accumulating
fp16 matmuls in 512-element chunks (the ISA's per-matmul PSUM-bank limit),
evacuated with the per-(i,row) bias b[o]*count added as a per-partition
scalar, then one output DMA per mh into y[mh, :, hl, i, :]
whose (hl, i) index order IS the row interleave, so the host just reshapes.
Edge stripes (s=0 row 0, s=3 row 3) split the evac in two because the bias
count varies on the outermost output rows; the split is structural on all
cores (SPMD), only the bias table data differs.
"""

import numpy as np

import concourse.bacc as bacc
import concourse.tile as tile
import concourse.mybir as mybir
from concourse.bass_utils import run_bass_kernel_spmd

B, C, H, WD = 2, 256, 64, 256
NT = 12                 # stacked tensors
NCORES = 8
HQ = H // 4             # 16 input rows per core
NS = 4                  # stripes per core
SR = HQ // NS           # 4 input rows per stripe
HOUT = NT * H           # 768

_F32 = mybir.dt.float32
_DT16 = mybir.dt.float16
_NP16 = mybir.dt.np(_DT16)

_NC_CACHE = {}


def build_nc(n_warm=14, first_split=True, xin_bufs=6, ob_bufs=8, ps_bufs=4, last_split=True,
             sp_tail_n=1, slb_ring="sp", mh_swap=False,
             mh0_3piece=False, sp_mh1_n=0, sp_mh0_n=0, p3r=("sp", "pool", "act"), split_ps=True, mh1b_act=True, m0r=("sp", "sp"), m1ar="pool", n_split=1, m0_rev=False):
    key = (n_warm, first_split, xin_bufs, ob_bufs, ps_bufs, last_split, sp_tail_n, slb_ring, mh_swap, mh0_3piece, sp_mh1_n, sp_mh0_n, p3r, split_ps, mh1b_act, m0r, m1ar, n_split, m0_rev)
    if key in _NC_CACHE:
        return _NC_CACHE[key]
    nc = bacc.Bacc("TRN2", target_bir_lowering=False)
    # x[p, s, i, kh, r*WD]: channel = kh*128 + p, input row = s*SR + r
    x_d = nc.declare_dram_parameter("x", [128, NS, NT, 2, SR * WD], _DT16, isOutput=False)
    w_d = nc.declare_dram_parameter("w", [2, 128, 2 * 128], _DT16, isOutput=False)
    bv_d = nc.declare_dram_parameter("bv", [2, 128, NT * HQ], _F32, isOutput=False)
    # y[mh, p, hl, i, w]: output channel = mh*128 + p, local out row = hl*NT + i
    y_d = nc.declare_dram_parameter("y", [2, 128, HQ, NT, WD], _DT16, isOutput=True)

    with tile.TileContext(nc) as tc:
        with (
            tc.tile_pool(name="const", bufs=1) as cpool,
            tc.tile_pool(name="xin", bufs=xin_bufs) as inpool,
            tc.tile_pool(name="obuf", bufs=ob_bufs) as outpool,
            tc.tile_pool(name="ps", bufs=ps_bufs, space="PSUM") as pspool,
        ):
            # consts all on the gpsimd ring, which is otherwise idle at the
            # head (ACT's head is occupied by the auto-inserted activation
            # table load; SP must start input tiles immediately); W per-kh,
            # kh0 first since it gates the first matmul
            # (the stationary matmul operand must be contiguous per partition,
            # so each [128,128] quadrant gets its own tile)
            wt = [
                [cpool.tile([128, 128], _DT16, name=f"w{kh}{mh}") for mh in range(2)]
                for kh in range(2)
            ]
            for mh in range(2):
                for kh in range(2):
                    nc.gpsimd.dma_start(
                        out=wt[kh][mh][:], in_=w_d[kh, :, mh * 128 : (mh + 1) * 128]
                    )
            # bias table on ACT (behind its table load, ready before the
            # first evac); keeping it off Pool keeps the W sems early, and
            # a late bv sem would stall the first evac -> PSUM recycling -> PE
            bvt = [cpool.tile([128, NT * HQ], _F32, name=f"bv{mh}") for mh in range(2)]
            for mh in range(2):
                nc.scalar.dma_start(out=bvt[mh][:], in_=bv_d[mh])

            # PE p-state warmup: pe_busy_start is sticky, so a burst of tiny
            # matmuls during the input-DMA fill window starts the 3us clock
            # ramp early and the real matmuls all run at full rate
            if n_warm:
                wscr = cpool.tile([128, 128], _DT16, name="wscr")
                zscr = cpool.tile([128, 64], _DT16, name="zscr")
                nc.vector.memset(wscr[:], 0.0)
                nc.vector.memset(zscr[:], 0.0)
                warm = pspool.tile([128, SR, WD], _F32, name="warm", tag="ps")
                for _ in range(n_warm):
                    nc.tensor.matmul(warm[:, 0, 0:64], wscr[:], zscr[:], start=True, stop=True)

            slb = {"sp": nc.sync, "act": nc.scalar, "pool": nc.gpsimd}[slb_ring]
            for s in range(NS):
                for i in range(NT):
                    if first_split and s == 0 and i == 0:
                        # first iteration: each kh half in its OWN tile so the
                        # first matmul waits only on its 790ns half-load (tile
                        # dependencies are tile-granular, and each DMA's
                        # completion sem costs +900ns)
                        xk = [
                            inpool.tile([128, SR * WD], _DT16, name=f"xk{kh}", tag=f"xk{kh}")
                            for kh in range(2)
                        ]
                        for kh in range(2):
                            nc.sync.dma_start(out=xk[kh][:], in_=x_d[:, s, i, kh])
                        xsl = lambda kh, a, b: xk[kh][:, a:b]
                    else:
                        xin = inpool.tile([128, 2, SR * WD], _DT16, name=f"x{s}_{i}", tag="xin")
                        nc.sync.dma_start(out=xin[:], in_=x_d[:, s, i])
                        xsl = lambda kh, a, b: xin[:, kh, a:b]
                    last_iter = s == NS - 1 and i == NT - 1
                    for mh in ((1, 0) if (mh_swap or last_iter) else (0, 1)):
                        ps = pspool.tile([128, SR, WD], _F32, name=f"ps{s}_{i}_{mh}", tag="ps")
                        # evac deps are tile-granular (an evac waits for ALL
                        # matmul chunks of its PSUM tile), so the very last
                        # tile splits its boundary row into a second tile:
                        # the rows0-2 evac can then start before the final mm
                        psb = (
                            pspool.tile([128, SR, WD], _F32, name=f"psb{s}_{i}_{mh}", tag="ps")
                            if (split_ps and s == NS - 1 and i >= NT - n_split)
                            else None
                        )
                        # bias b[o]*count as per-partition scalar; count is
                        # uniform within an op, so edge stripes split the
                        # boundary row off (count ramps on the outer 11 rows).
                        # s=NS-1 also splits the matmuls/DMA so the kernel's
                        # drain tail ends on a 1-row sliver
                        if s == 0:
                            mm_parts = [(0, SR)]
                            parts = [(0, 1, 0), (1, SR, 1)]
                        elif s == NS - 1:
                            if mh0_3piece and last_iter and mh == 0:
                                mm_parts = [(0, SR - 1), (SR - 1, SR)]
                                parts = [(0, 2, s * SR), (2, 3, s * SR + 2), (SR - 1, SR, HQ - 1)]
                            else:
                                mm_parts = [(0, SR - 1), (SR - 1, SR)]
                                parts = [(0, SR - 1, s * SR), (SR - 1, SR, HQ - 1)]
                        else:
                            mm_parts = [(0, SR)]
                            parts = [(0, SR, s * SR)]
                        # the ISA caps a matmul's moving/out free size at 512
                        # elements (one PSUM bank), so emit 2-row chunks
                        for r0, r1 in mm_parts:
                            for c0 in range(r0, r1, 2):
                                c1 = min(c0 + 2, r1)
                                pdst = psb if (psb is not None and c0 >= SR - 1) else ps
                                for j, kh in enumerate((0, 1)):
                                    nc.tensor.matmul(
                                        pdst[:, c0:c1], wt[kh][mh][:],
                                        xsl(kh, c0 * WD, c1 * WD),
                                        start=(j == 0), stop=(j == 1),
                                    )
                        ob = outpool.tile([128, SR, WD], _DT16, name=f"ob{s}_{i}_{mh}", tag=f"ob{mh}")
                        for r0, r1, hl in parts:
                            col = i * HQ + hl
                            psrc = psb if (psb is not None and r0 >= SR - 1) else ps
                            # the last iteration's boundary-row evacs swap
                            # engines: mh1's goes to ACT (idle in that window)
                            # and mh0's to DVE (free by then), so neither
                            # queues behind the other tail evacs
                            if last_iter and r0 >= SR - 1 and mh1b_act:
                                use_act = mh == 1
                            else:
                                use_act = mh == 0
                            if use_act:
                                nc.scalar.activation(
                                    ob[:, r0:r1],
                                    psrc[:, r0:r1],
                                    mybir.ActivationFunctionType.Identity,
                                    bias=bvt[mh][:, col : col + 1],
                                )
                            else:
                                nc.vector.tensor_scalar_add(
                                    ob[:, r0:r1],
                                    psrc[:, r0:r1],
                                    bvt[mh][:, col : col + 1],
                                )
                        if last_split and last_iter:
                            # tail: pipeline the final pieces across rings
                            # (the SP ring is drained by now and HWDGE has a
                            # shorter completion-sem lag than SWDGE)
                            if mh0_3piece and mh == 0:
                                r3 = {"sp": nc.sync, "act": nc.scalar, "pool": nc.gpsimd}
                                pieces = [(0, 2, r3[p3r[0]]), (2, 3, r3[p3r[1]]), (3, 4, r3[p3r[2]])]
                            elif mh == 0:
                                r3 = {"sp": nc.sync, "act": nc.scalar, "pool": nc.gpsimd}
                                pieces = [(0, SR - 1, r3[m0r[0]]), (SR - 1, SR, r3[m0r[1]])]
                                if m0_rev:
                                    pieces.reverse()
                            else:
                                r3 = {"sp": nc.sync, "act": nc.scalar, "pool": nc.gpsimd}
                                pieces = [(0, SR - 1, r3[m1ar]), (SR - 1, SR, slb)]
                            for r0, r1, eng in pieces:
                                eng.dma_start(
                                    out=y_d[mh, :, s * SR + r0 : s * SR + r1, i, :],
                                    in_=ob[:, r0:r1],
                                )
                        else:
                            # the gpsimd/SWDGE completion sem lags ~1.1us
                            # behind the transfer, so the tail-most regular
                            # outputs go on the drained SP ring instead
                            k = (sp_mh1_n if mh == 1 else sp_mh0_n)
                            out_eng = (
                                nc.sync
                                if (last_split and s == NS - 1
                                    and (i >= NT - sp_tail_n or i >= NT - 1 - k))
                                else nc.gpsimd
                            )
                            out_eng.dma_start(
                                out=y_d[mh, :, s * SR : (s + 1) * SR, i, :],
                                in_=ob[:],
                            )
    nc.finalize()
    _NC_CACHE[key] = nc
    return nc


def _counts() -> np.ndarray:
    """count[r] for output row r (conv-transpose bias multiplicity)."""
    r = np.arange(HOUT)
    return (np.minimum(11, r) - np.maximum(0, r - (HOUT - NT)) + 1).astype(np.float32)


def shard_inputs(inputs: dict) -> list[dict]:
    xs = [np.asarray(inputs[f"x{i}"], dtype=np.float32) for i in range(NT)]
    w = np.asarray(inputs["W"], dtype=np.float32)
    b = np.asarray(inputs["b"], dtype=np.float32)
    counts = _counts()
    # w[kh, k, mh, m] = W[kh*128+k, mh*128+m]
    wp = np.ascontiguousarray(
        w.reshape(2, 128, 2 * 128).astype(_NP16)
    )
    in_maps = []
    for cid in range(NCORES):
        b_idx, hq = divmod(cid, 4)
        h0 = hq * HQ
        # x[p, s, i, kh, r*WD] = x_i[b, kh*128+p, h0+s*SR+r, w]
        xp = np.empty((128, NS, NT, 2, SR * WD), dtype=_NP16)
        for i in range(NT):
            blk = xs[i][b_idx, :, h0 : h0 + HQ, :]  # [256, 16, 256]
            blk = blk.reshape(2, 128, NS, SR * WD)  # [kh, p, s, rw]
            xp[:, :, i] = blk.transpose(1, 2, 0, 3).astype(_NP16)
        # bv[mh, m, i*HQ + hl] = b[mh*128+m] * count(12*(h0+hl) + i)
        i_idx = np.arange(NT)[:, None]
        hl_idx = np.arange(HQ)[None, :]
        cnt = counts[NT * (h0 + hl_idx) + i_idx].reshape(NT * HQ)  # [192]
        bv = (b.reshape(2, 128)[:, :, None] * cnt[None, None, :]).astype(np.float32)
        in_maps.append({"x": xp, "w": wp, "bv": bv})
    return in_maps


def gather_outputs(results: list[dict]) -> np.ndarray:
    out = np.empty((B, C, HOUT, WD), dtype=np.float32)
    for cid in range(NCORES):
        b_idx, hq = divmod(cid, 4)
        h0 = hq * HQ
        # y[mh, p, hl, i, w] -> rows hl*NT+i: exactly the interleave order
        y = np.asarray(results[cid]["y"]).reshape(C, HQ * NT, WD)
        out[b_idx, :, NT * h0 : NT * (h0 + HQ), :] = y.astype(np.float32)
    return out


def kernel(**inputs) -> np.ndarray:
    nc = build_nc()
    in_maps = shard_inputs(inputs)
    res = run_bass_kernel_spmd(nc, in_maps, core_ids=list(range(NCORES)))
    return gather_outputs(res.results)
